# revision 29
# baseline (speedup 1.0000x reference)
"""Trainium2 Bass kernel for ComputeAlignmentError.

reference math:
    t[b,i,j,k] = dot(coords[b,i] - origin[b,j], E[b,j,k])   (per pred/true)
    out[b,i,j] = sqrt(sum_k (t_pred - t_true)^2 + 1e-8)

Quadratic-form formulation (K=50 full outer-product packing):
    u[i]    = [pred_coords[i] (3), true_coords[i] (3), -1]          (7)
    A[j]    = rows_k [E_pred[j,k] (3), -E_true[j,k] (3), c[j,k]]    (3x7)
    err2[i,j] = sum_{f,g} u_f u_g G49[j, f*7+g],  G49 = sum_k A_kf A_kg
    row 49 carries eps: M[49]=1, G[49]=1e-8  ->  err2 + 1e-8 in PSUM
    out[i,j]  = sqrt(PSUM)          (pure sqrt on ACT, no bias)

v2 structure:
  - G-side transposes take chunk PAIRS: transpose([128 j, 128 = ch(2)x64fg])
    lands chunk 2c's G^T at partitions 0-63 and chunk 2c+1's at 64-127 --
    exactly the h0/h64 row-tiled stream layout, no dup DMA.
  - products run as h0/h64 row-group pairs: h0 streams even chunks' j
    blocks, h64 odd chunks'; PSUM pair [128, 512+512]; one double-width
    ACT sqrt drains both with a j-interleaving AP into contiguous OTB.
  - products use float32r (PE single-pass fp32, 1 cyc/col at >=256 free)
    behind PROD_FP32R; fall back to fp32 LOW_HIGH if precision fails.
  - input DMAs split across 4 engine queues; 8 output DMAs (512KB)
    issued per sqrt completion on alternating queues.

Sharding: output rows i split across 8 cores; frame prep replicated.
"""

import numpy as np

B = 2            # batches
N = 2048         # n residues
NCORES = 8
RPC = N // NCORES          # rows per core per batch = 256
P = 128                    # partitions
NCH = N // P               # j-chunks per batch = 16
G64 = 2 * B * NCH          # (t, b, c) groups = 64
HB = G64 // 2              # per-engine half (t split) = 32
NG = 4                     # i-tile groups per core (B * RPC/P)
KC = 50                    # contraction: 49 products + eps row
EPS_NORM2 = 1e-16          # added under sqrt for normalize (== max vs 1e-8)
EPS_ERR = 1e-8
PROD_FP32R = False         # float32r single-pass PE mode: ~15 mantissa
                           # bits -- fails the cancellation error budget

_cache = {}


def _build():
    import concourse.bass as bass
    import concourse.bacc as bacc
    import concourse.tile as tile
    import concourse.mybir as mybir
    from concourse.masks import make_identity

    F32 = mybir.dt.float32
    F32R = mybir.dt.float32r
    MUL = mybir.AluOpType.mult
    ADD = mybir.AluOpType.add
    SUB = mybir.AluOpType.subtract
    SQRT = mybir.ActivationFunctionType.Sqrt

    nc = bacc.Bacc("TRN2", target_bir_lowering=False, debug=False,
                   num_devices=NCORES)

    pc_d = nc.dram_tensor("pc", [B, RPC, 3], F32, kind="ExternalInput")
    tc_d = nc.dram_tensor("tcrd", [B, RPC, 3], F32, kind="ExternalInput")
    pf_d = nc.dram_tensor("pf", [B, N, 3, 3], F32, kind="ExternalInput")
    tf_d = nc.dram_tensor("tf", [B, N, 3, 3], F32, kind="ExternalInput")
    out_d = nc.dram_tensor("out", [B, RPC, N], F32, kind="ExternalOutput")

    def v(tileap, offset_elems, dims):
        """AP view: keep partition dim of `tileap`, custom free dims."""
        return bass.AP(tensor=tileap.tensor,
                       offset=tileap.offset + offset_elems,
                       ap=[tileap.ap[0]] + dims)

    MMDT = F32R if PROD_FP32R else F32

    with tile.TileContext(nc) as tc:
        with (
            tc.tile_pool(name="consts", bufs=1) as consts,
            tc.tile_pool(name="prep", bufs=1) as prep,
            tc.tile_pool(name="gkp", bufs=2) as gkp,
            tc.tile_pool(name="mside", bufs=1) as mside,
            tc.tile_pool(name="gt", bufs=1) as gtp,
            tc.tile_pool(name="ps_t", bufs=2, space="PSUM") as ps_t,
            tc.tile_pool(name="ps_mm", bufs=2, space="PSUM") as ps_mm,
            tc.tile_pool(name="outp", bufs=1) as outp,
        ):
            ident = consts.tile([P, P], F32)
            make_identity(nc, ident[:])
            warm = consts.tile([P, 2], F32)
            nc.vector.memset(warm[:], 1.0)
            epsn = consts.tile([P, 1], F32)
            nc.vector.memset(epsn[:], EPS_NORM2)
            # force the Sqrt ACT table load early, off the critical path
            nc.scalar.activation(out=warm[:, 1:2], in_=warm[:, 0:1],
                                 func=SQRT)

            # ---- input DMAs, split across 4 engine queues ----------------
            # frames: partition p <- frame j = ch*128 + p (stride-9 gather);
            # F[jp, t, b, ch, e]; e = d*3 + pt (pt fastest)
            F = prep.tile([P, 2, B, NCH, 9], F32)

            def fpiece(eng, t, b, c0, nch):
                dram = (pf_d, tf_d)[t]
                src = bass.AP(tensor=dram, offset=(b * N + c0 * P) * 9,
                              ap=[[9, P], [P * 9, nch], [1, 9]])
                dst = v(F[:], ((t * B + b) * NCH + c0) * 9,
                        [[9, nch], [1, 9]])
                eng.dma_start(out=dst, in_=src)

            fpiece(nc.sync, 0, 0, 0, 16)       # pf b0
            fpiece(nc.scalar, 0, 1, 0, 16)     # pf b1
            fpiece(nc.gpsimd, 1, 0, 0, 16)     # tf b0
            fpiece(nc.sync, 1, 1, 0, 16)       # tf b1

            # coords: all 4 i-tiles in one DMA per tensor
            UU = prep.tile([P, NG, 7], F32)
            for ci, (dram, eng) in enumerate(((pc_d, nc.scalar),
                                              (tc_d, nc.gpsimd))):
                src = bass.AP(tensor=dram, offset=0,
                              ap=[[3, P], [P * 3, NG], [1, 3]])
                dst = v(UU[:], ci * 3, [[7, NG], [1, 3]])
                eng.dma_start(out=dst, in_=src)
            nc.gpsimd.memset(v(UU[:], 6, [[7, NG], [1, 1]]), -1.0)

            # ---- M side: M49[i, grp, f*7+g] = u_f * u_g  (+ eps row + dup)
            M49 = mside.tile([P, NG, P], F32)
            m_ap = M49[:]
            for base, eng in ((0, nc.vector), (64, nc.gpsimd)):
                eng.tensor_tensor(
                    out=v(m_ap, base, [[P, NG], [7, 7], [1, 7]]),
                    in0=v(UU[:], 0, [[7, NG], [1, 7], [0, 7]]),
                    in1=v(UU[:], 0, [[7, NG], [0, 7], [1, 7]]),
                    op=MUL)
            # col 49 = 1.0 (pairs with G eps row); cols 50-63 unread garbage
            nc.gpsimd.memset(v(m_ap, 49, [[P, NG], [64, 2], [1, 1]]), 1.0)

            MTT = []
            for g in range(NG):
                tp_m = ps_t.tile([P, P], F32, name=f"tpm{g}", tag="tpm")
                nc.tensor.transpose(tp_m[:], M49[:, g, :], ident[:])
                MT = mside.tile([P, P], MMDT, name=f"mt{g}", tag=f"mt{g}")
                if g % 2 == 0:
                    nc.vector.tensor_copy(out=MT[:], in_=tp_m[:])
                else:
                    nc.scalar.copy(out=MT[:], in_=tp_m[:])
                MTT.append(MT)

            # ---- frame bases, vectorized over g = (t, b, c); t split across
            # vector (t=0 / pred) and gpsimd (t=1 / true) engines.  The true
            # half's subtractions are operand-swapped => e-basis negated.
            fap = F[:]
            ENG = (nc.vector, nc.gpsimd)

            # W12[p, w(2), g, 3]: w=0 -> a-b (pt0-pt1), w=1 -> c-b (pt2-pt1)
            # frames free layout per group is (d, pt): pt stride 1, d stride 3
            W12 = prep.tile([P, 2, G64, 3], F32)
            for h in range(2):
                pts = v(fap, h * HB * 9, [[2, 2], [9, HB], [3, 3]])
                ptb = v(fap, h * HB * 9 + 1, [[0, 2], [9, HB], [3, 3]])
                ow = v(W12[:], h * HB * 3, [[G64 * 3, 2], [3, HB], [1, 3]])
                if h == 0:
                    ENG[h].tensor_tensor(out=ow, in0=pts, in1=ptb, op=SUB)
                else:          # swapped: negated basis for the true half
                    ENG[h].tensor_tensor(out=ow, in0=ptb, in1=pts, op=SUB)

            def normalize2(X, dst_maker, nm, precise=False):
                """X [P, 2, G64, 3] -> dst = X / sqrt(|X|^2 + 1e-16).
                precise=True uses the exact reciprocal: required when the
                normalized outputs feed a cancelling subtraction (w1n-w2n
                can be ~1e-4 for near-parallel frame legs, so a 1e-5
                reciprocal error there becomes a ~1e-1 direction error).
                reduce/sqrt/recip run PER HALF so the h0 (pred) chain
                proceeds while h1 still waits on its later frame DMA."""
                SQ = prep.tile([P, 2, G64, 3], F32, name=f"sq{nm}", tag=f"sq{nm}")
                SS = prep.tile([P, 2, G64], F32, name=f"ss{nm}", tag=f"ss{nm}")
                RCP = prep.tile([P, 2, G64], F32, name=f"rc{nm}", tag=f"rc{nm}")
                NRM = prep.tile([P, 2, G64], F32, name=f"nr{nm}", tag=f"nr{nm}")
                for h in range(2):
                    iv = v(X[:], h * HB * 3, [[G64 * 3, 2], [3, HB], [1, 3]])
                    ov = v(SQ[:], h * HB * 3, [[G64 * 3, 2], [3, HB], [1, 3]])
                    ENG[h].tensor_tensor(out=ov, in0=iv, in1=iv, op=MUL)
                    nc.vector.tensor_reduce(
                        out=v(SS[:], h * HB, [[G64, 2], [1, HB]]),
                        in_=v(SQ[:], h * HB * 3,
                              [[G64 * 3, 2], [3, HB], [1, 3]]),
                        axis=mybir.AxisListType.X, op=ADD)
                    nc.scalar.activation(
                        out=v(NRM[:], h * HB, [[G64, 2], [1, HB]]),
                        in_=v(SS[:], h * HB, [[G64, 2], [1, HB]]),
                        func=SQRT, bias=epsn[:], scale=1.0)
                    rcp_o = v(RCP[:], h * HB, [[G64, 2], [1, HB]])
                    nrm_i = v(NRM[:], h * HB, [[G64, 2], [1, HB]])
                    if precise:
                        nc.vector.reciprocal(rcp_o, nrm_i)
                    else:
                        nc.vector.reciprocal_approx_fast(rcp_o, nrm_i)
                    iv = v(X[:], h * HB * 3, [[G64 * 3, 2], [3, HB], [1, 3]])
                    rv = v(RCP[:], h * HB, [[G64, 2], [1, HB], [0, 3]])
                    ENG[h].tensor_tensor(out=dst_maker(h), in0=iv, in1=rv,
                                         op=MUL)
                return RCP

            W12N = prep.tile([P, 2, G64, 3], F32)
            normalize2(W12, lambda h: v(W12N[:], h * HB * 3,
                                        [[G64 * 3, 2], [3, HB], [1, 3]]), "w",
                       precise=True)

            # SD[p, s(2), g, 3]: s=0 -> w1n+w2n, s=1 -> w2n-w1n
            SD = prep.tile([P, 2, G64, 3], F32)
            for h in range(2):
                ENG[h].tensor_tensor(
                    out=v(SD[:], h * HB * 3, [[3, HB], [1, 3]]),
                    in0=v(W12N[:], h * HB * 3, [[3, HB], [1, 3]]),
                    in1=v(W12N[:], G64 * 3 + h * HB * 3, [[3, HB], [1, 3]]),
                    op=ADD)
            for h in range(2):
                ENG[h].tensor_tensor(
                    out=v(SD[:], G64 * 3 + h * HB * 3, [[3, HB], [1, 3]]),
                    in0=v(W12N[:], G64 * 3 + h * HB * 3, [[3, HB], [1, 3]]),
                    in1=v(W12N[:], h * HB * 3, [[3, HB], [1, 3]]),
                    op=SUB)

            # ---- A[bc, k, f7] written in place: [Ep | -Et | c]
            A = prep.tile([P, B * NCH, 3, 7], F32)
            a_ap = A[:]

            def sd_dst(h):
                # (s, ch, d) -> A[ch, k=s, f=3h+d]
                return v(a_ap, 3 * h, [[7, 2], [21, HB], [1, 3]])

            RCS = normalize2(SD, sd_dst, "s")

            # e3 = e1 x e2 into A[ch, 2, 3h+d]; true half swapped (negated)
            TA = prep.tile([P, 2, 3, HB], F32)
            TB = prep.tile([P, 2, 3, HB], F32)
            for h in range(2):
                for x in range(3):
                    y, z = (x + 1) % 3, (x + 2) % 3
                    ENG[h].tensor_tensor(
                        out=TA[:, h, x],
                        in0=v(a_ap, 3 * h + y, [[21, HB]]),
                        in1=v(a_ap, 7 + 3 * h + z, [[21, HB]]), op=MUL)
                    ENG[h].tensor_tensor(
                        out=TB[:, h, x],
                        in0=v(a_ap, 3 * h + z, [[21, HB]]),
                        in1=v(a_ap, 7 + 3 * h + y, [[21, HB]]), op=MUL)
                e3 = v(a_ap, 14 + 3 * h, [[1, 3], [21, HB]])
                if h == 0:
                    ENG[h].tensor_tensor(out=e3, in0=TA[:, h], in1=TB[:, h],
                                         op=SUB)
                else:
                    ENG[h].tensor_tensor(out=e3, in0=TB[:, h], in1=TA[:, h],
                                         op=SUB)

            # ---- c column: OC_h = sum_d E_h[k,d] * origin_h[d]
            # (E rows already carry the true-half negation)
            OP = prep.tile([P, G64, 3, 3], F32)
            op_ap = OP[:]
            for h in range(2):
                ENG[h].tensor_tensor(
                    out=v(op_ap, h * HB * 9, [[9, HB], [3, 3], [1, 3]]),
                    in0=v(a_ap, 3 * h, [[21, HB], [7, 3], [1, 3]]),
                    in1=v(fap, h * HB * 9 + 1, [[9, HB], [0, 3], [3, 3]]),
                    op=MUL)
            OC = prep.tile([P, G64, 3], F32)
            oc_ap = OC[:]
            nc.vector.tensor_reduce(out=OC[:], in_=OP[:],
                                    axis=mybir.AxisListType.X, op=ADD)
            # A[:, :, 6] = OC_pred + OC_negtrue = o_p.Ep - o_t.Et = c
            nc.gpsimd.tensor_tensor(
                out=v(a_ap, 6, [[21, B * NCH], [7, 3]]),
                in0=v(oc_ap, 0, [[3, B * NCH], [1, 3]]),
                in1=v(oc_ap, B * NCH * 3, [[3, B * NCH], [1, 3]]),
                op=ADD)

            # private copy of A for the gpsimd half of the G49 build --
            # both engines hammering one A tile cost ~2x on those ops
            ACPY = prep.tile([P, B * NCH, 3, 7], F32)
            nc.gpsimd.tensor_copy(out=ACPY[:], in_=A[:])
            ac_ap = ACPY[:]

            # ---- per batch: G49, pair-transposes, products, sqrt, DMA out
            GT = [gtp.tile([P, N], MMDT, name=f"gt{b}", tag=f"gt{b}")
                  for b in range(B)]
            OTB = outp.tile([P, NG, N], F32)
            otb_ap = OTB[:]
            DRAIN_ENGS = (nc.vector, nc.scalar)
            OUT_QS = (nc.sync, nc.scalar)
            for b in range(B):
                G49p = gkp.tile([P, NCH, 64], F32, name=f"g49_{b}", tag="g49")
                g_ap = G49p[:]
                nc.gpsimd.memset(v(g_ap, 49, [[64, NCH], [1, 1]]), EPS_ERR)
                GK = gkp.tile([P, NCH, 3, 49], F32, name=f"gk{b}", tag="gk")
                GKS = gkp.tile([P, NCH, 49], F32, name=f"gks{b}", tag="gks")
                gk_ap, gks_ap = GK[:], GKS[:]
                # half hf entirely on one engine: no cross-engine deps inside
                for hf in range(2):
                    eng = ENG[hf]
                    src_ap = a_ap if hf == 0 else ac_ap
                    co = hf * 8
                    aoff = (b * NCH + co) * 21
                    for k in range(3):
                        eng.tensor_tensor(
                            out=v(gk_ap, co * 147 + k * 49,
                                  [[147, 8], [7, 7], [1, 7]]),
                            in0=v(src_ap, aoff + k * 7,
                                  [[21, 8], [1, 7], [0, 7]]),
                            in1=v(src_ap, aoff + k * 7,
                                  [[21, 8], [0, 7], [1, 7]]),
                            op=MUL)
                    eng.tensor_tensor(
                        out=v(gks_ap, co * 49, [[49, 8], [1, 49]]),
                        in0=v(gk_ap, co * 147, [[147, 8], [1, 49]]),
                        in1=v(gk_ap, co * 147 + 49, [[147, 8], [1, 49]]),
                        op=ADD)
                    eng.tensor_tensor(
                        out=v(g_ap, co * 64, [[64, 8], [1, 49]]),
                        in0=v(gks_ap, co * 49, [[49, 8], [1, 49]]),
                        in1=v(gk_ap, co * 147 + 98, [[147, 8], [1, 49]]),
                        op=ADD)

                gt_ap = GT[b][:]
                # chunk-pair transposes: [128 j, 128 = 2 chunks x 64fg]
                # -> partitions 0-63 = even chunk G^T, 64-127 = odd chunk
                for cp in range(8):
                    tpg = ps_t.tile([P, P], F32, name=f"tpg{b}_{cp}",
                                    tag="tpg")
                    nc.tensor.transpose(
                        tpg[:], v(g_ap, cp * P, [[1, P]]), ident[:])
                    deng = DRAIN_ENGS[cp % 2]
                    if deng is nc.scalar:
                        deng.copy(out=GT[b][:, cp * P:(cp + 1) * P],
                                  in_=tpg[:])
                    else:
                        deng.tensor_copy(out=GT[b][:, cp * P:(cp + 1) * P],
                                         in_=tpg[:])

                    # after each half-N's 4 chunk-pairs: products + sqrt + DMA
                    if cp % 4 == 3:
                        hn = cp // 4
                        cols = slice(hn * 512, (hn + 1) * 512)
                        for s in range(2):
                            g = b * 2 + s
                            pairp = ps_mm.tile([P, 1024],
                                               F32, name=f"pp{b}{hn}{s}",
                                               tag="pp")
                            for hg in range(2):
                                base = 64 * hg
                                nc.tensor.matmul(
                                    pairp[:, hg * 512:(hg + 1) * 512],
                                    MTT[g][base:base + KC, :],
                                    GT[b][base:base + KC, cols],
                                    start=True, stop=True)
                            # interleave j blocks back to contiguous:
                            # j_local = blk*256 + hg*128 + c
                            nc.scalar.activation(
                                out=v(otb_ap, g * N + hn * 1024,
                                      [[128, 2], [256, 4], [1, 128]]),
                                in_=v(pairp[:], 0,
                                      [[512, 2], [128, 4], [1, 128]]),
                                func=SQRT)
                            dst = bass.AP(
                                tensor=out_d,
                                offset=(b * RPC + s * P) * N + hn * 1024,
                                ap=[[N, P], [1, 1024]])
                            OUT_QS[(b * 4 + hn * 2 + s) % 2].dma_start(
                                out=dst,
                                in_=v(otb_ap, g * N + hn * 1024,
                                      [[1, 1024]]))

    nc.compile()
    return nc


def _get_nc():
    if "nc" not in _cache:
        _cache["nc"] = _build()
    return _cache["nc"]


def _in_maps(pred_coords, true_coords, pred_frames, true_frames):
    pc = np.ascontiguousarray(pred_coords, dtype=np.float32)
    tcd = np.ascontiguousarray(true_coords, dtype=np.float32)
    pf = np.ascontiguousarray(pred_frames, dtype=np.float32)
    tf = np.ascontiguousarray(true_frames, dtype=np.float32)
    maps = []
    for c in range(NCORES):
        sl = slice(c * RPC, (c + 1) * RPC)
        maps.append({
            "pc": np.ascontiguousarray(pc[:, sl]),
            "tcrd": np.ascontiguousarray(tcd[:, sl]),
            "pf": pf,
            "tf": tf,
        })
    return maps


def _assemble(results):
    full = np.empty((B, N, N), dtype=np.float32)
    for c in range(NCORES):
        full[:, c * RPC:(c + 1) * RPC, :] = results[c]["out"]
    return full


def run_hw(trace=False, **inputs):
    from concourse.bass_utils import run_bass_kernel_spmd
    nc = _get_nc()
    res = run_bass_kernel_spmd(nc, _in_maps(**inputs), list(range(NCORES)),
                               trace=trace)
    return _assemble(res.results), res


def kernel(**inputs):
    out, _ = run_hw(trace=False, **inputs)
    return out


# revision 30
# speedup vs baseline: 1.0355x; 1.0355x over previous
"""Trainium2 Bass kernel for ComputeAlignmentError.

reference math:
    t[b,i,j,k] = dot(coords[b,i] - origin[b,j], E[b,j,k])   (per pred/true)
    out[b,i,j] = sqrt(sum_k (t_pred - t_true)^2 + 1e-8)

Quadratic-form formulation (K=50 full outer-product packing):
    u[i]    = [pred_coords[i] (3), true_coords[i] (3), -1]          (7)
    A[j]    = rows_k [E_pred[j,k] (3), -E_true[j,k] (3), c[j,k]]    (3x7)
    err2[i,j] = sum_{f,g} u_f u_g G49[j, f*7+g],  G49 = sum_k A_kf A_kg
    row 49 carries eps: M[49]=1, G[49]=1e-8  ->  err2 + 1e-8 in PSUM
    out[i,j]  = sqrt(PSUM)          (pure sqrt on ACT, no bias)

Structure (v4, ~64-70us vs 75us baseline; HW run-to-run ~+-3us):
  - G-side transposes take chunk PAIRS: transpose([128 j, 128 = ch(2)x64fg])
    lands chunk 2c's G^T at partitions 0-63 and chunk 2c+1's at 64-127 --
    exactly the h0/h64 row-tiled stream layout, no dup DMA.
  - products run as h0/h64 row-group pairs: h0 streams even chunks' j
    blocks, h64 odd chunks'; PSUM pair [128, 512+512]; one double-width
    ACT sqrt drains both with a j-interleaving AP into contiguous OTB;
    8 early 512KB output DMAs on alternating queues.
  - normalize reduce/sqrt/recip run PER HALF so the pred chain advances
    while the true half waits on its later frame DMA piece.
  - the gpsimd half of the G49 build reads a private copy of A (the two
    DVE-class engines contend ~2x when hammering one tile).
  - input DMAs split across sync/scalar HWDGE + gpsimd queues.
  - float32r notes (measured): single-pass f32r keeps ~15 mantissa bits
    -> fails the err^2 cancellation budget (min out 0.0144 needs ~2e-6
    absolute in err^2).  A 3-pass f32r hi/lo split (MhGh+MhGl+MlGh) IS
    accurate (7.4e-4) but the doubled transposes/drains made it slower
    at the mid PE p-state; products stay fp32 LOW_HIGH (kernel_hilo.py
    keeps the variant).

Sharding: output rows i split across 8 cores; frame prep replicated.
"""

import numpy as np

B = 2            # batches
N = 2048         # n residues
NCORES = 8
RPC = N // NCORES          # rows per core per batch = 256
P = 128                    # partitions
NCH = N // P               # j-chunks per batch = 16
G64 = 2 * B * NCH          # (t, b, c) groups = 64
HB = G64 // 2              # per-engine half (t split) = 32
NG = 4                     # i-tile groups per core (B * RPC/P)
KC = 50                    # contraction: 49 products + eps row
EPS_NORM2 = 1e-16          # added under sqrt for normalize (== max vs 1e-8)
EPS_ERR = 1e-8
PROD_FP32R = False         # float32r single-pass PE mode: ~15 mantissa
                           # bits -- fails the cancellation error budget

_cache = {}


def _build():
    import concourse.bass as bass
    import concourse.bacc as bacc
    import concourse.tile as tile
    import concourse.mybir as mybir
    from concourse.masks import make_identity

    F32 = mybir.dt.float32
    F32R = mybir.dt.float32r
    MUL = mybir.AluOpType.mult
    ADD = mybir.AluOpType.add
    SUB = mybir.AluOpType.subtract
    SQRT = mybir.ActivationFunctionType.Sqrt

    nc = bacc.Bacc("TRN2", target_bir_lowering=False, debug=False,
                   num_devices=NCORES)

    pc_d = nc.dram_tensor("pc", [B, RPC, 3], F32, kind="ExternalInput")
    tc_d = nc.dram_tensor("tcrd", [B, RPC, 3], F32, kind="ExternalInput")
    pf_d = nc.dram_tensor("pf", [B, N, 3, 3], F32, kind="ExternalInput")
    tf_d = nc.dram_tensor("tf", [B, N, 3, 3], F32, kind="ExternalInput")
    out_d = nc.dram_tensor("out", [B, RPC, N], F32, kind="ExternalOutput")

    def v(tileap, offset_elems, dims):
        """AP view: keep partition dim of `tileap`, custom free dims."""
        return bass.AP(tensor=tileap.tensor,
                       offset=tileap.offset + offset_elems,
                       ap=[tileap.ap[0]] + dims)

    MMDT = F32R if PROD_FP32R else F32

    with tile.TileContext(nc) as tc:
        with (
            tc.tile_pool(name="consts", bufs=1) as consts,
            tc.tile_pool(name="prep", bufs=1) as prep,
            tc.tile_pool(name="gkp", bufs=2) as gkp,
            tc.tile_pool(name="mside", bufs=1) as mside,
            tc.tile_pool(name="gt", bufs=1) as gtp,
            tc.tile_pool(name="ps_t", bufs=2, space="PSUM") as ps_t,
            tc.tile_pool(name="ps_mm", bufs=2, space="PSUM") as ps_mm,
            tc.tile_pool(name="outp", bufs=1) as outp,
        ):
            ident = consts.tile([P, P], F32)
            make_identity(nc, ident[:])
            warm = consts.tile([P, 2], F32)
            nc.vector.memset(warm[:], 1.0)
            epsn = consts.tile([P, 1], F32)
            nc.vector.memset(epsn[:], EPS_NORM2)
            # force the Sqrt ACT table load early, off the critical path
            nc.scalar.activation(out=warm[:, 1:2], in_=warm[:, 0:1],
                                 func=SQRT)

            # ---- input DMAs, split across 4 engine queues ----------------
            # frames: partition p <- frame j = ch*128 + p (stride-9 gather);
            # F[jp, t, b, ch, e]; e = d*3 + pt (pt fastest)
            F = prep.tile([P, 2, B, NCH, 9], F32)

            def fpiece(eng, t, b, c0, nch):
                dram = (pf_d, tf_d)[t]
                src = bass.AP(tensor=dram, offset=(b * N + c0 * P) * 9,
                              ap=[[9, P], [P * 9, nch], [1, 9]])
                dst = v(F[:], ((t * B + b) * NCH + c0) * 9,
                        [[9, nch], [1, 9]])
                eng.dma_start(out=dst, in_=src)

            fpiece(nc.sync, 0, 0, 0, 16)       # pf b0
            fpiece(nc.scalar, 0, 1, 0, 16)     # pf b1
            fpiece(nc.gpsimd, 1, 0, 0, 16)     # tf b0
            fpiece(nc.sync, 1, 1, 0, 16)       # tf b1

            # coords: all 4 i-tiles in one DMA per tensor
            UU = prep.tile([P, NG, 7], F32)
            for ci, (dram, eng) in enumerate(((pc_d, nc.scalar),
                                              (tc_d, nc.gpsimd))):
                src = bass.AP(tensor=dram, offset=0,
                              ap=[[3, P], [P * 3, NG], [1, 3]])
                dst = v(UU[:], ci * 3, [[7, NG], [1, 3]])
                eng.dma_start(out=dst, in_=src)
            nc.gpsimd.memset(v(UU[:], 6, [[7, NG], [1, 1]]), -1.0)

            # ---- M side: M49[i, grp, f*7+g] = u_f * u_g  (+ eps row + dup)
            M49 = mside.tile([P, NG, P], F32)
            m_ap = M49[:]
            for base, eng in ((0, nc.vector), (64, nc.gpsimd)):
                eng.tensor_tensor(
                    out=v(m_ap, base, [[P, NG], [7, 7], [1, 7]]),
                    in0=v(UU[:], 0, [[7, NG], [1, 7], [0, 7]]),
                    in1=v(UU[:], 0, [[7, NG], [0, 7], [1, 7]]),
                    op=MUL)
            # col 49 = 1.0 (pairs with G eps row); cols 50-63 unread garbage
            nc.gpsimd.memset(v(m_ap, 49, [[P, NG], [64, 2], [1, 1]]), 1.0)

            MTT = []
            for g in range(NG):
                tp_m = ps_t.tile([P, P], F32, name=f"tpm{g}", tag="tpm")
                nc.tensor.transpose(tp_m[:], M49[:, g, :], ident[:])
                MT = mside.tile([P, P], MMDT, name=f"mt{g}", tag=f"mt{g}")
                if g % 2 == 0:
                    nc.vector.tensor_copy(out=MT[:], in_=tp_m[:])
                else:
                    nc.scalar.copy(out=MT[:], in_=tp_m[:])
                MTT.append(MT)

            # ---- frame bases, vectorized over g = (t, b, c); t split across
            # vector (t=0 / pred) and gpsimd (t=1 / true) engines.  The true
            # half's subtractions are operand-swapped => e-basis negated.
            fap = F[:]
            ENG = (nc.vector, nc.gpsimd)

            # W12[p, w(2), g, 3]: w=0 -> a-b (pt0-pt1), w=1 -> c-b (pt2-pt1)
            # frames free layout per group is (d, pt): pt stride 1, d stride 3
            W12 = prep.tile([P, 2, G64, 3], F32)
            for h in range(2):
                pts = v(fap, h * HB * 9, [[2, 2], [9, HB], [3, 3]])
                ptb = v(fap, h * HB * 9 + 1, [[0, 2], [9, HB], [3, 3]])
                ow = v(W12[:], h * HB * 3, [[G64 * 3, 2], [3, HB], [1, 3]])
                if h == 0:
                    ENG[h].tensor_tensor(out=ow, in0=pts, in1=ptb, op=SUB)
                else:          # swapped: negated basis for the true half
                    ENG[h].tensor_tensor(out=ow, in0=ptb, in1=pts, op=SUB)

            def normalize2(X, dst_maker, nm, precise=False):
                """X [P, 2, G64, 3] -> dst = X / sqrt(|X|^2 + 1e-16).
                precise=True uses the exact reciprocal: required when the
                normalized outputs feed a cancelling subtraction (w1n-w2n
                can be ~1e-4 for near-parallel frame legs, so a 1e-5
                reciprocal error there becomes a ~1e-1 direction error).
                reduce/sqrt/recip run PER HALF so the h0 (pred) chain
                proceeds while h1 still waits on its later frame DMA."""
                SQ = prep.tile([P, 2, G64, 3], F32, name=f"sq{nm}", tag=f"sq{nm}")
                SS = prep.tile([P, 2, G64], F32, name=f"ss{nm}", tag=f"ss{nm}")
                RCP = prep.tile([P, 2, G64], F32, name=f"rc{nm}", tag=f"rc{nm}")
                NRM = prep.tile([P, 2, G64], F32, name=f"nr{nm}", tag=f"nr{nm}")
                for h in range(2):
                    iv = v(X[:], h * HB * 3, [[G64 * 3, 2], [3, HB], [1, 3]])
                    ov = v(SQ[:], h * HB * 3, [[G64 * 3, 2], [3, HB], [1, 3]])
                    ENG[h].tensor_tensor(out=ov, in0=iv, in1=iv, op=MUL)
                    nc.vector.tensor_reduce(
                        out=v(SS[:], h * HB, [[G64, 2], [1, HB]]),
                        in_=v(SQ[:], h * HB * 3,
                              [[G64 * 3, 2], [3, HB], [1, 3]]),
                        axis=mybir.AxisListType.X, op=ADD)
                    nc.scalar.activation(
                        out=v(NRM[:], h * HB, [[G64, 2], [1, HB]]),
                        in_=v(SS[:], h * HB, [[G64, 2], [1, HB]]),
                        func=SQRT, bias=epsn[:], scale=1.0)
                    rcp_o = v(RCP[:], h * HB, [[G64, 2], [1, HB]])
                    nrm_i = v(NRM[:], h * HB, [[G64, 2], [1, HB]])
                    if precise:
                        nc.vector.reciprocal(rcp_o, nrm_i)
                    else:
                        nc.vector.reciprocal_approx_fast(rcp_o, nrm_i)
                    iv = v(X[:], h * HB * 3, [[G64 * 3, 2], [3, HB], [1, 3]])
                    rv = v(RCP[:], h * HB, [[G64, 2], [1, HB], [0, 3]])
                    ENG[h].tensor_tensor(out=dst_maker(h), in0=iv, in1=rv,
                                         op=MUL)
                return RCP

            W12N = prep.tile([P, 2, G64, 3], F32)
            normalize2(W12, lambda h: v(W12N[:], h * HB * 3,
                                        [[G64 * 3, 2], [3, HB], [1, 3]]), "w",
                       precise=True)

            # SD[p, s(2), g, 3]: s=0 -> w1n+w2n, s=1 -> w2n-w1n
            SD = prep.tile([P, 2, G64, 3], F32)
            for h in range(2):
                ENG[h].tensor_tensor(
                    out=v(SD[:], h * HB * 3, [[3, HB], [1, 3]]),
                    in0=v(W12N[:], h * HB * 3, [[3, HB], [1, 3]]),
                    in1=v(W12N[:], G64 * 3 + h * HB * 3, [[3, HB], [1, 3]]),
                    op=ADD)
            for h in range(2):
                ENG[h].tensor_tensor(
                    out=v(SD[:], G64 * 3 + h * HB * 3, [[3, HB], [1, 3]]),
                    in0=v(W12N[:], G64 * 3 + h * HB * 3, [[3, HB], [1, 3]]),
                    in1=v(W12N[:], h * HB * 3, [[3, HB], [1, 3]]),
                    op=SUB)

            # ---- A[bc, k, f7] written in place: [Ep | -Et | c]
            A = prep.tile([P, B * NCH, 3, 7], F32)
            a_ap = A[:]

            def sd_dst(h):
                # (s, ch, d) -> A[ch, k=s, f=3h+d]
                return v(a_ap, 3 * h, [[7, 2], [21, HB], [1, 3]])

            RCS = normalize2(SD, sd_dst, "s")

            # e3 = e1 x e2 into A[ch, 2, 3h+d]; true half swapped (negated)
            TA = prep.tile([P, 2, 3, HB], F32)
            TB = prep.tile([P, 2, 3, HB], F32)
            for h in range(2):
                for x in range(3):
                    y, z = (x + 1) % 3, (x + 2) % 3
                    ENG[h].tensor_tensor(
                        out=TA[:, h, x],
                        in0=v(a_ap, 3 * h + y, [[21, HB]]),
                        in1=v(a_ap, 7 + 3 * h + z, [[21, HB]]), op=MUL)
                    ENG[h].tensor_tensor(
                        out=TB[:, h, x],
                        in0=v(a_ap, 3 * h + z, [[21, HB]]),
                        in1=v(a_ap, 7 + 3 * h + y, [[21, HB]]), op=MUL)
                e3 = v(a_ap, 14 + 3 * h, [[1, 3], [21, HB]])
                if h == 0:
                    ENG[h].tensor_tensor(out=e3, in0=TA[:, h], in1=TB[:, h],
                                         op=SUB)
                else:
                    ENG[h].tensor_tensor(out=e3, in0=TB[:, h], in1=TA[:, h],
                                         op=SUB)

            # ---- c column: OC_h = sum_d E_h[k,d] * origin_h[d]
            # (E rows already carry the true-half negation)
            OP = prep.tile([P, G64, 3, 3], F32)
            op_ap = OP[:]
            for h in range(2):
                ENG[h].tensor_tensor(
                    out=v(op_ap, h * HB * 9, [[9, HB], [3, 3], [1, 3]]),
                    in0=v(a_ap, 3 * h, [[21, HB], [7, 3], [1, 3]]),
                    in1=v(fap, h * HB * 9 + 1, [[9, HB], [0, 3], [3, 3]]),
                    op=MUL)
            OC = prep.tile([P, G64, 3], F32)
            oc_ap = OC[:]
            nc.vector.tensor_reduce(out=OC[:], in_=OP[:],
                                    axis=mybir.AxisListType.X, op=ADD)
            # A[:, :, 6] = OC_pred + OC_negtrue = o_p.Ep - o_t.Et = c
            nc.gpsimd.tensor_tensor(
                out=v(a_ap, 6, [[21, B * NCH], [7, 3]]),
                in0=v(oc_ap, 0, [[3, B * NCH], [1, 3]]),
                in1=v(oc_ap, B * NCH * 3, [[3, B * NCH], [1, 3]]),
                op=ADD)

            # private copy of A for the gpsimd half of the G49 build --
            # both engines hammering one A tile cost ~2x on those ops
            ACPY = prep.tile([P, B * NCH, 3, 7], F32)
            nc.gpsimd.tensor_copy(out=ACPY[:], in_=A[:])
            ac_ap = ACPY[:]

            # ---- per batch: G49, pair-transposes, products, sqrt, DMA out
            GT = [gtp.tile([P, N], MMDT, name=f"gt{b}", tag=f"gt{b}")
                  for b in range(B)]
            OTB = outp.tile([P, NG, N], F32)
            otb_ap = OTB[:]
            DRAIN_ENGS = (nc.vector, nc.scalar)
            OUT_QS = (nc.sync, nc.scalar)
            for b in range(B):
                G49p = gkp.tile([P, NCH, 64], F32, name=f"g49_{b}", tag="g49")
                g_ap = G49p[:]
                nc.gpsimd.memset(v(g_ap, 49, [[64, NCH], [1, 1]]), EPS_ERR)
                GK = gkp.tile([P, NCH, 3, 49], F32, name=f"gk{b}", tag="gk")
                GKS = gkp.tile([P, NCH, 49], F32, name=f"gks{b}", tag="gks")
                gk_ap, gks_ap = GK[:], GKS[:]
                # half hf entirely on one engine: no cross-engine deps inside
                for hf in range(2):
                    eng = ENG[hf]
                    src_ap = a_ap if hf == 0 else ac_ap
                    co = hf * 8
                    aoff = (b * NCH + co) * 21
                    for k in range(3):
                        eng.tensor_tensor(
                            out=v(gk_ap, co * 147 + k * 49,
                                  [[147, 8], [7, 7], [1, 7]]),
                            in0=v(src_ap, aoff + k * 7,
                                  [[21, 8], [1, 7], [0, 7]]),
                            in1=v(src_ap, aoff + k * 7,
                                  [[21, 8], [0, 7], [1, 7]]),
                            op=MUL)
                    eng.tensor_tensor(
                        out=v(gks_ap, co * 49, [[49, 8], [1, 49]]),
                        in0=v(gk_ap, co * 147, [[147, 8], [1, 49]]),
                        in1=v(gk_ap, co * 147 + 49, [[147, 8], [1, 49]]),
                        op=ADD)
                    eng.tensor_tensor(
                        out=v(g_ap, co * 64, [[64, 8], [1, 49]]),
                        in0=v(gks_ap, co * 49, [[49, 8], [1, 49]]),
                        in1=v(gk_ap, co * 147 + 98, [[147, 8], [1, 49]]),
                        op=ADD)

                gt_ap = GT[b][:]
                # chunk-pair transposes: [128 j, 128 = 2 chunks x 64fg]
                # -> partitions 0-63 = even chunk G^T, 64-127 = odd chunk
                for cp in range(8):
                    tpg = ps_t.tile([P, P], F32, name=f"tpg{b}_{cp}",
                                    tag="tpg")
                    nc.tensor.transpose(
                        tpg[:], v(g_ap, cp * P, [[1, P]]), ident[:])
                    deng = DRAIN_ENGS[cp % 2]
                    if deng is nc.scalar:
                        deng.copy(out=GT[b][:, cp * P:(cp + 1) * P],
                                  in_=tpg[:])
                    else:
                        deng.tensor_copy(out=GT[b][:, cp * P:(cp + 1) * P],
                                         in_=tpg[:])

                    # after each half-N's 4 chunk-pairs: products + sqrt + DMA
                    if cp % 4 == 3:
                        hn = cp // 4
                        cols = slice(hn * 512, (hn + 1) * 512)
                        for s in range(2):
                            g = b * 2 + s
                            pairp = ps_mm.tile([P, 1024],
                                               F32, name=f"pp{b}{hn}{s}",
                                               tag="pp")
                            for hg in range(2):
                                base = 64 * hg
                                nc.tensor.matmul(
                                    pairp[:, hg * 512:(hg + 1) * 512],
                                    MTT[g][base:base + KC, :],
                                    GT[b][base:base + KC, cols],
                                    start=True, stop=True)
                            # interleave j blocks back to contiguous:
                            # j_local = blk*256 + hg*128 + c
                            nc.scalar.activation(
                                out=v(otb_ap, g * N + hn * 1024,
                                      [[128, 2], [256, 4], [1, 128]]),
                                in_=v(pairp[:], 0,
                                      [[512, 2], [128, 4], [1, 128]]),
                                func=SQRT)
                            dst = bass.AP(
                                tensor=out_d,
                                offset=(b * RPC + s * P) * N + hn * 1024,
                                ap=[[N, P], [1, 1024]])
                            OUT_QS[(b * 4 + hn * 2 + s) % 2].dma_start(
                                out=dst,
                                in_=v(otb_ap, g * N + hn * 1024,
                                      [[1, 1024]]))

    nc.compile()
    return nc


def _get_nc():
    if "nc" not in _cache:
        _cache["nc"] = _build()
    return _cache["nc"]


def _in_maps(pred_coords, true_coords, pred_frames, true_frames):
    pc = np.ascontiguousarray(pred_coords, dtype=np.float32)
    tcd = np.ascontiguousarray(true_coords, dtype=np.float32)
    pf = np.ascontiguousarray(pred_frames, dtype=np.float32)
    tf = np.ascontiguousarray(true_frames, dtype=np.float32)
    maps = []
    for c in range(NCORES):
        sl = slice(c * RPC, (c + 1) * RPC)
        maps.append({
            "pc": np.ascontiguousarray(pc[:, sl]),
            "tcrd": np.ascontiguousarray(tcd[:, sl]),
            "pf": pf,
            "tf": tf,
        })
    return maps


def _assemble(results):
    full = np.empty((B, N, N), dtype=np.float32)
    for c in range(NCORES):
        full[:, c * RPC:(c + 1) * RPC, :] = results[c]["out"]
    return full


def run_hw(trace=False, **inputs):
    from concourse.bass_utils import run_bass_kernel_spmd
    nc = _get_nc()
    res = run_bass_kernel_spmd(nc, _in_maps(**inputs), list(range(NCORES)),
                               trace=trace)
    return _assemble(res.results), res


def kernel(**inputs):
    out, _ = run_hw(trace=False, **inputs)
    return out
